# revision 28
# baseline (speedup 1.0000x reference)
"""GCN layer on 8 Trainium2 NeuronCores.

out = relu(D^{-1/2} (A+I) D^{-1/2} x W^T + b),  N=8192, D=512, A symmetric binary.

Sharding (1-D graph partition, rank c owns nodes [c*1024, (c+1)*1024)):
  - A+I is symmetric, so the row-block (A+I)[own, :] each core must aggregate
    equals the column slab (A+I)[:, own] transposed: each core is fed the
    natural column slab, which is exactly the [K, N] layout the PE wants.
  - All normalization is folded on the host (deg is a cheap host reduction):
      slab' = (A+I)[:, own] * d_own^{-1/2}[col]
      y     = d^{-1/2}[:, None] * x               (replicated)
    so the device does exactly two matmuls and a relu:
      hT[feat, own] = y^T @ slab'   (contract over all 8192 nodes)
      out[own, :]   = relu(hT^T @ W^T + b)
    No collectives, no cast-DMAs, no on-device degree pass.
  - The aggregation matmul runs bf16(y) x fp8e4(slab'): binary adjacency
    scaled by d^{-1/2} fits fp8 within bf16-equivalent accuracy while halving
    slab HBM traffic (PE upconverts both operands; rate is bf16's).
  - Streams: slab chunks on the SP HWDGE queue, y/wt on the ACT HWDGE queue,
    out rows on the SWDGE (gpsimd) queue so stores never block input FIFOs.
"""

import numpy as np

N = 8192
D = 512
NCORES = 8
B = N // NCORES          # 1024 nodes per core
P = 128
KT = N // P              # 64 k-tiles of 128 rows
SCH = 8                  # slab chunks (8 k-tiles each)
SKPC = KT // SCH         # k-tiles per chunk
YCH = 4                  # y chunks (16 k-tiles each)
YKPC = KT // YCH

# variant knobs (A/B-able; defaults = current best)
SLAB_FP8 = True          # slab in fp8e4 (mixed-dtype matmul vs bf16 slab)
OUT_SWDGE = True         # out DMA on gpsimd queue (vs sync HWDGE)
QUEUE_BAL = False        # split slab/y across both HWDGE queues evenly
NCHUNKS = (16, 8)        # (slab chunks, y chunks) per rep
PACK_KT = 24             # k-tiles aggregated as fp8 DoubleRow pairs (0..64,
                         # multiple of KT//NCHUNKS[1]); trades accuracy
                         # (e4m3 y on those rows) for 4 MMs saved per pair
YSC = 16.0               # y pre-scale (exact bf16 shift; e4m3 range fit)

_cache = {}


def _build(with_bias: bool, ar_chunks: int = 1, reps: int = 1,
           serialize_reps: bool = False, skip_collectives: bool = False,
           num_devices: int = NCORES, mm_n1024: bool = False):
    import concourse.tile as tile
    from concourse import bacc, mybir

    f32 = mybir.dt.float32
    bf16 = mybir.dt.bfloat16
    sdt = mybir.dt.float8e4 if SLAB_FP8 else bf16

    nc = bacc.Bacc("TRN2", target_bir_lowering=False, debug=False,
                   num_devices=num_devices)

    f8 = mybir.dt.float8e4
    dr = mybir.MatmulPerfMode.DoubleRow
    assert PACK_KT == 0 or SLAB_FP8, "DoubleRow packing needs the fp8 slab"

    slab_d = nc.dram_tensor("slab", [N, B], sdt, kind="ExternalInput").ap()
    y_d = nc.dram_tensor("y", [N, D], bf16, kind="ExternalInput").ap()
    if PACK_KT:
        y8_d = nc.dram_tensor("y8", [PACK_KT * P, D], f8,
                              kind="ExternalInput").ap()
    wt_d = nc.dram_tensor("wt", [D, D], bf16, kind="ExternalInput").ap()
    if SLAB_FP8:
        # fp8 slab stays binary-exact; d_own^{-1/2} applied on evacuation
        dvo_d = nc.dram_tensor("dvo", [P, SCH], f32, kind="ExternalInput").ap()
    if with_bias:
        bb_d = nc.dram_tensor("bb", [P, D], f32, kind="ExternalInput").ap()
    out_d = nc.dram_tensor("out", [B, D], f32, kind="ExternalOutput").ap()
    out_r = out_d.rearrange("(m p) f -> p m f", p=P)

    with tile.TileContext(nc) as tc:
        with tc.tile_pool(name="slab", bufs=1) as slab_pool, \
             tc.tile_pool(name="y", bufs=1) as y_pool, \
             tc.tile_pool(name="small", bufs=1) as small, \
             tc.tile_pool(name="osb", bufs=1) as osb_pool, \
             tc.tile_pool(name="psum", bufs=1, space="PSUM") as psum_pool:
          sch_n, ych_n = NCHUNKS
          skpc, ykpc = KT // sch_n, KT // ych_n
          sslots = 4 if sch_n >= 8 else 2
          yslots = 4 if ych_n >= 4 else 2
          for _rep in range(reps):
            par = _rep % 2
            # ---- input streams: slab split per own-half so the h=0
            # aggregation pass, its projection and stores overlap the h=1
            # pass; y/y8 double-buffered resident (read by both passes) ----
            slab_sb = {}           # (h, ch) -> tile [P, skpc, 512]
            for h in range(2):
                for ch in range(sch_n):
                    t = slab_pool.tile([P, skpc, 512], sdt,
                                       name=f"slab{h}_{ch}",
                                       tag=f"sl{h}_{ch % sslots}")
                    src = slab_d[ch * (skpc * P):(ch + 1) * (skpc * P),
                                 h * 512:(h + 1) * 512]
                    nc.sync.dma_start(t[:],
                                      src.rearrange("(n p) f -> p n f", p=P))
                    slab_sb[(h, ch)] = t
            wt_sb = small.tile([P, D // P, D], bf16, name="wt_sb", tag="wt",
                               bufs=2)
            nc.scalar.dma_start(wt_sb[:],
                                wt_d.rearrange("(kf p) f -> p kf f", p=P))
            if SLAB_FP8:
                dvo_sb = small.tile([P, SCH], f32, name="dvo_sb", tag="dvo",
                                    bufs=2)
                nc.scalar.dma_start(dvo_sb[:], dvo_d[:])
            if with_bias:
                bb = small.tile([P, D], f32, name="bb_sb", tag="bb", bufs=2)
                nc.scalar.dma_start(bb[:], bb_d[:])
            pack_ch = PACK_KT // ykpc        # y chunks covered by packed rows
            assert PACK_KT % ykpc == 0 and PACK_KT % 2 == 0 and skpc % 2 == 0
            y_sb = []
            for ch in range(ych_n):
                if ch < pack_ch:
                    y_sb.append(None)        # packed rows use y8 instead
                    continue
                t = y_pool.tile([P, ykpc, D], bf16, name=f"y{ch}",
                                tag=f"y{par}_{ch}")
                src = y_d[ch * (ykpc * P):(ch + 1) * (ykpc * P), :]
                nc.scalar.dma_start(t[:],
                                    src.rearrange("(n p) f -> p n f", p=P))
                y_sb.append(t)
            y8_sb = []
            for ch in range(pack_ch):
                t = y_pool.tile([P, ykpc, D], f8, name=f"y8{ch}",
                                tag=f"y8{par}_{ch}")
                src = y8_d[ch * (ykpc * P):(ch + 1) * (ykpc * P), :]
                nc.scalar.dma_start(t[:],
                                    src.rearrange("(n p) f -> p n f", p=P))
                y8_sb.append(t)

            # ---- two aggregation passes (one per own-half): the h=0 half's
            # PSUM drain, projection and stores overlap the h=1 pass ----
            hT_sb = small.tile([P, 4, B], bf16, name="hT_sb", tag="hT",
                               bufs=2)

            def mm1_pass(h, ps):
                for pt in range(PACK_KT // 2):
                    kt0 = 2 * pt
                    sch, si = divmod(kt0, skpc)
                    ych8, yi8 = divmod(kt0, ykpc)
                    for mf in range(4):
                        nc.tensor.matmul(
                            ps[mf],
                            lhsT=y8_sb[ych8][:, yi8:yi8 + 2,
                                             mf * P:(mf + 1) * P],
                            rhs=slab_sb[(h, sch)][:, si:si + 2, :],
                            start=(pt == 0),
                            stop=(PACK_KT == KT and pt == PACK_KT // 2 - 1),
                            perf_mode=dr)
                for kt in range(PACK_KT, KT):
                    sch, si = divmod(kt, skpc)
                    ych, yi = divmod(kt, ykpc)
                    for mf in range(4):
                        nc.tensor.matmul(
                            ps[mf],
                            lhsT=y_sb[ych][:, yi, mf * P:(mf + 1) * P],
                            rhs=slab_sb[(h, sch)][:, si, :],
                            start=(kt == PACK_KT and PACK_KT == 0),
                            stop=(kt == KT - 1))

            def evac(h, ps):
                for mf in range(4):
                    dst = hT_sb[:, mf, h * 512:(h + 1) * 512]
                    if mf % 2 == 0:
                        nc.scalar.copy(dst, ps[mf][:])
                    else:
                        nc.vector.tensor_copy(dst, ps[mf][:])

            def mm2(m):
                o_ps = psum_pool.tile([P, D], f32, name=f"ops_{m}",
                                      tag=f"ps_{m}")
                for kf in range(4):
                    nc.tensor.matmul(o_ps,
                                     lhsT=hT_sb[:, kf, m * P:(m + 1) * P],
                                     rhs=wt_sb[:, kf, :],
                                     start=(kf == 0), stop=(kf == 3))
                o_sb = osb_pool.tile([P, D], f32, name=f"osb{m}",
                                     tag=f"osb{m % 2}", bufs=2)
                if SLAB_FP8:
                    if with_bias:
                        nc.vector.tensor_scalar_mul(o_sb[:], o_ps[:],
                                                    dvo_sb[:, m:m + 1])
                        nc.vector.tensor_add(o_sb[:], o_sb[:], bb[:])
                        nc.vector.tensor_scalar_max(o_sb[:], o_sb[:], 0.0)
                    elif m % 2 == 0:
                        # relu(o_ps * dvo) in one ScalarE op; odd on DVE
                        nc.scalar.activation(
                            o_sb[:], o_ps[:],
                            mybir.ActivationFunctionType.Relu,
                            scale=dvo_sb[:, m:m + 1])
                    else:
                        nc.vector.tensor_scalar(o_sb[:], o_ps[:],
                                                dvo_sb[:, m:m + 1], 0.0,
                                                mybir.AluOpType.mult,
                                                mybir.AluOpType.max)
                elif with_bias:
                    nc.vector.tensor_add(o_sb[:], o_ps[:], bb[:])
                    nc.vector.tensor_scalar_max(o_sb[:], o_sb[:], 0.0)
                else:
                    nc.vector.tensor_scalar_max(o_sb[:], o_ps[:], 0.0)
                oeng = nc.gpsimd if OUT_SWDGE else nc.sync
                oeng.dma_start(out_r[:, m, :], o_sb[:])

            ps0 = [psum_pool.tile([P, 512], f32, name=f"ps0_{mf}",
                                  tag=f"ps_{mf}") for mf in range(4)]
            mm1_pass(0, ps0)
            evac(0, ps0)
            ps1 = [psum_pool.tile([P, 512], f32, name=f"ps1_{mf}",
                                  tag=f"ps_{4 + mf}") for mf in range(4)]
            mm1_pass(1, ps1)
            for m in range(4):
                mm2(m)            # deps: evac(0) only — overlaps pass 1
            evac(1, ps1)
            for m in range(4, SCH):
                mm2(m)

    nc.compile()
    return nc


def _prep_in_maps(x, A, W, b, with_bias):
    import ml_dtypes
    bf16 = ml_dtypes.bfloat16
    sdt = ml_dtypes.float8_e4m3 if SLAB_FP8 else bf16

    deg = A.astype(np.float32).sum(axis=1) + 1.0          # A binary, +I
    dv = (1.0 / np.sqrt(deg)).astype(np.float32)
    ysc = YSC if SLAB_FP8 else 1.0        # exact bf16 shift; undone via dvo
    ys = ysc * dv[:, None] * x.astype(np.float32)
    y = ys.astype(bf16)
    y8 = ys[:PACK_KT * P].astype(ml_dtypes.float8_e4m3) if PACK_KT else None
    wt = np.ascontiguousarray(W.astype(np.float32).T).astype(bf16)
    in_maps = []
    for c in range(NCORES):
        own = slice(c * B, (c + 1) * B)
        sl = np.array(A[:, own], dtype=np.float32)
        sl[np.arange(c * B, (c + 1) * B), np.arange(B)] += 1.0  # fold +I
        if SLAB_FP8:
            # keep the slab binary (exact in e4m3); scale rows on evacuation
            m = {"slab": sl.astype(sdt), "y": y, "wt": wt,
                 "dvo": np.ascontiguousarray(
                     (dv[own] / ysc).reshape(SCH, P).T)}
            if PACK_KT:
                m["y8"] = y8
        else:
            sl *= dv[own][None, :]                        # fold d_own^{-1/2}
            m = {"slab": sl.astype(sdt), "y": y, "wt": wt}
        if with_bias:
            m["bb"] = np.ascontiguousarray(
                np.broadcast_to(b.astype(np.float32), (P, D)))
        in_maps.append(m)
    return in_maps


def get_compiled(with_bias, ar_chunks=1, reps=1, serialize_reps=False,
                 skip_collectives=False, num_devices=NCORES, mm_n1024=False):
    key = (SLAB_FP8, OUT_SWDGE, QUEUE_BAL, NCHUNKS, PACK_KT, with_bias,
           ar_chunks, reps, serialize_reps, skip_collectives, num_devices,
           mm_n1024)
    if key not in _cache:
        _cache[key] = _build(with_bias, ar_chunks, reps, serialize_reps,
                             skip_collectives, num_devices, mm_n1024)
    return _cache[key]


def kernel(x, A, W, b):
    from concourse import bass_utils

    with_bias = bool(np.any(b))
    nc = get_compiled(with_bias)
    in_maps = _prep_in_maps(x, A, W, b, with_bias)
    try:
        res = bass_utils.run_bass_kernel_spmd(nc, in_maps,
                                              core_ids=list(range(NCORES)))
    except Exception:
        # the shared terminal occasionally wedges (NRT_EXEC_UNIT_UNRECOVERABLE
        # from a prior session); it auto-resets after ~1 min
        import time
        time.sleep(75)
        res = bass_utils.run_bass_kernel_spmd(nc, in_maps,
                                              core_ids=list(range(NCORES)))
    out = np.concatenate([res.results[c]["out"] for c in range(NCORES)], axis=0)
    return out.astype(np.float32)
